# revision 1
# baseline (speedup 1.0000x reference)
"""Trainium2 Bass kernel for the stacked-LSTM model (nn_Model2_16904991277618).

Model: LSTM-A(64->40, return_sequences) -> LSTM-B(40->40, last) over T=1024,
plus a small dense tail on `feat`, concat, 3 dense layers -> sigmoid [B,1].

Strategy: data-parallel over batch (B=512 -> 64 rows/core on 8 cores),
feature-major layout on chip so the sequential scan maps onto the tensor
engine with zero per-step transposes. Host pre-transposes seq to per-core
[F+1, T, Bc] (bf16, ones row for bias) so each step's matmul rhs is an SBUF
slice.

Per-cell gate layout (partition starts must be 0/64; SBUF-SBUF operand pairs
must share bases, PSUM-SBUF may mix):
  zz  PSUM [128, 2*Bc]: cols 0:Bc    -> i @ rows 0:40,  f @ rows 64:104
                        cols Bc:2Bc  -> o @ rows 0:40,  g @ rows 64:104
  gp  PSUM [128, 2*Bc] = Sigmoid(zz) in ONE activation (g slot is unused
      garbage); tanh(g) and tanh(c) are separate activations.
  DVE reads i/f/o straight from PSUM (mixed-base legal vs SBUF operands).
"""

import functools
import os
import sys

import numpy as np

for _p in ("/opt/trn_rl_repo", "/root/.axon_site/_ro/trn_rl_repo"):
    if os.path.isdir(_p) and _p not in sys.path:
        sys.path.insert(0, _p)

import ml_dtypes  # noqa: E402

import concourse.bass as bass  # noqa: E402
import concourse.bacc as bacc  # noqa: E402
import concourse.mybir as mybir  # noqa: E402
import concourse.tile as tile  # noqa: E402
from concourse.bass_utils import run_bass_kernel_spmd  # noqa: E402

F32 = mybir.dt.float32
BF16 = mybir.dt.bfloat16
AF = mybir.ActivationFunctionType
OP = mybir.AluOpType

NCORES = 8
H = 40
D = 10
F = 64

# gate column ranges in the reference [*, 4H] weight matrices
_I, _Fg, _G, _O = slice(0, 40), slice(40, 80), slice(80, 120), slice(120, 160)


def _bf(x):
    return np.ascontiguousarray(x, dtype=ml_dtypes.bfloat16)


def _f32c(x):
    return np.ascontiguousarray(x, dtype=np.float32)


def _wpair(w, b, s0, s1, krows, bias_row):
    """Build lhsT [krows(+1), 128] with gate s0 at cols 0:40, s1 at 64:104.

    If bias_row, append one row carrying the bias (rhs must supply ones).
    """
    w = np.asarray(w, np.float32)
    b = np.asarray(b, np.float32)
    k = w.shape[0]
    out = np.zeros((k + (1 if bias_row else 0), 128), np.float32)
    out[:k, 0:40] = w[:, s0]
    out[:k, 64:104] = w[:, s1]
    if bias_row:
        out[k, 0:40] = b[s0]
        out[k, 64:104] = b[s1]
    return _bf(out)


def _build_program(T, BC, use_bias_b=False):
    CHUNK_T = min(T, 128)
    n_chunks = T // CHUNK_T
    assert n_chunks * CHUNK_T == T
    BC2 = 2 * BC

    nc = bacc.Bacc("TRN2", debug=False, target_bir_lowering=False,
                   num_devices=NCORES)

    def din(name, shape, dt):
        return nc.dram_tensor(name, list(shape), dt, kind="ExternalInput").ap()

    xt = din("xt", (n_chunks, F + 1, CHUNK_T * BC), BF16)
    featT = din("featT", (F, BC), BF16)
    d_in = {
        "wa_x_if": din("wa_x_if", (F + 1, 128), BF16),
        "wa_x_og": din("wa_x_og", (F + 1, 128), BF16),
        "wa_h_if": din("wa_h_if", (H, 128), BF16),
        "wa_h_og": din("wa_h_og", (H, 128), BF16),
        "wb_k_if": din("wb_k_if", (H, 128), BF16),
        "wb_k_og": din("wb_k_og", (H, 128), BF16),
        "wb_r_if": din("wb_r_if", (H, 128), BF16),
        "wb_r_og": din("wb_r_og", (H, 128), BF16),
        "bb_if": din("bb_if", (1, 128), BF16),
        "bb_og": din("bb_og", (1, 128), BF16),
        "wg": din("wg", (F, D), BF16),
        "wh": din("wh", (D, D), BF16),
        "wc": din("wc", (74, 2 * D), BF16),
        "wd": din("wd", (2 * D, D), BF16),
        "wo": din("wo", (D, 1), BF16),
        "bg": din("bg", (D, 1), F32),
        "bh": din("bh", (D, 1), F32),
        "bc2": din("bc2", (2 * D, 1), F32),
        "bd": din("bd", (D, 1), F32),
        "bo": din("bo", (1, 1), F32),
    }

    out_dram = nc.dram_tensor("out", [1, BC], F32, kind="ExternalOutput").ap()

    from contextlib import ExitStack

    with tile.TileContext(nc) as tc:
        with ExitStack() as ctx:
            wpool = ctx.enter_context(tc.tile_pool(name="w", bufs=1))
            xpool = ctx.enter_context(tc.tile_pool(name="x", bufs=1))
            gpool = ctx.enter_context(tc.tile_pool(name="g", bufs=3))
            hpool = ctx.enter_context(tc.tile_pool(name="h", bufs=4))
            cpool = ctx.enter_context(tc.tile_pool(name="c", bufs=3))
            tpool = ctx.enter_context(tc.tile_pool(name="t", bufs=3))
            spool = ctx.enter_context(tc.tile_pool(name="s", bufs=1))
            psum = ctx.enter_context(tc.tile_pool(name="ps", bufs=2,
                                                  space="PSUM"))

            W = {}
            for nm, src in d_in.items():
                t = wpool.tile(list(src.shape), src.dtype, name=f"w_{nm}")
                nc.sync.dma_start(t[:], src[:])
                W[nm] = t
            ftile = wpool.tile([F, BC], BF16, name="w_featT")
            nc.sync.dma_start(ftile[:], featT[:])
            ones = wpool.tile([1, BC], BF16, name="ones")
            nc.gpsimd.memset(ones[:], 1.0)

            xch = []
            for ci in range(n_chunks):
                xc = xpool.tile([F + 1, CHUNK_T * BC], BF16, name=f"xc{ci}",
                                tag=f"xc{ci}")
                nc.sync.dma_start(xc[:], xt[ci])
                xch.append(xc)

            ha = hpool.tile([H, BC], BF16, name="ha0", tag="ha")
            hb = hpool.tile([H, BC], BF16, name="hb0", tag="hb")
            ca = cpool.tile([H, BC], F32, name="ca0", tag="ca")
            cb = cpool.tile([H, BC], F32, name="cb0", tag="cb")
            for z in (ha, hb, ca, cb):
                nc.gpsimd.memset(z[:], 0.0)


            def cell_front(which, h_in, c_in, xr):
                """MMs + gate activations + c-update for one LSTM step.

                zz/gp [128, 2BC]: cols 0:BC = (i@0, f@64), BC:2BC = (o@0,
                g@64). Returns (gp, c_new, tg_inst) for the tail phase.
                """
                zz = psum.tile([128, BC2], F32, name=f"zz_{which}",
                               tag=f"zz{which}")
                zif, zog = zz[:, 0:BC], zz[:, BC:BC2]
                if which == "a":
                    nc.tensor.matmul(zif, W["wa_x_if"][:], xr,
                                     start=True, stop=False)
                    nc.tensor.matmul(zog, W["wa_x_og"][:], xr,
                                     start=True, stop=False)
                    nc.tensor.matmul(zif, W["wa_h_if"][:], h_in[:],
                                     start=False, stop=True)
                    nc.tensor.matmul(zog, W["wa_h_og"][:], h_in[:],
                                     start=False, stop=True)
                else:
                    if use_bias_b:
                        nc.tensor.matmul(zif, W["bb_if"][:], ones[:],
                                         start=True, stop=False)
                        nc.tensor.matmul(zog, W["bb_og"][:], ones[:],
                                         start=True, stop=False)
                    nc.tensor.matmul(zif, W["wb_k_if"][:], xr,
                                     start=not use_bias_b, stop=False)
                    nc.tensor.matmul(zog, W["wb_k_og"][:], xr,
                                     start=not use_bias_b, stop=False)
                    nc.tensor.matmul(zif, W["wb_r_if"][:], h_in[:],
                                     start=False, stop=True)
                    nc.tensor.matmul(zog, W["wb_r_og"][:], h_in[:],
                                     start=False, stop=True)

                gp = psum.tile([128, BC2], F32, name=f"gp_{which}",
                               tag=f"gp{which}")
                i_sig = nc.scalar.activation(gp[:], zz[:], AF.Sigmoid)
                tg = gpool.tile([H, BC], BF16, name=f"tg_{which}",
                                tag=f"tg{which}")
                i_tg = nc.scalar.activation(tg[:], zz[64:104, BC:BC2],
                                            AF.Tanh)

                p = tpool.tile([H, BC], F32, name=f"p_{which}", tag=f"p{which}")
                i_p = nc.vector.tensor_tensor(p[:], gp[64:104, 0:BC], c_in[:],
                                              OP.mult)
                m = tpool.tile([H, BC], F32, name=f"m_{which}", tag=f"m{which}")
                nc.vector.tensor_tensor(m[:], gp[0:40, 0:BC], tg[:], OP.mult)
                c_new = cpool.tile([H, BC], F32, name=f"c_{which}",
                                   tag=f"c{which}")
                i_c = nc.vector.tensor_tensor(c_new[:], m[:], p[:], OP.add)
                return gp, c_new, i_sig, i_tg, i_p, i_c

            def cell_tail(which, gp, c_new):
                """tanh(c) + h for one LSTM step."""
                tch = gpool.tile([H, BC], BF16, name=f"tc_{which}",
                                 tag=f"tc{which}")
                nc.scalar.activation(tch[:], c_new[:], AF.Tanh)
                h_new = hpool.tile([H, BC], BF16, name=f"h_{which}",
                                   tag=f"h{which}")
                i_h = nc.vector.tensor_tensor(h_new[:], gp[0:40, BC:BC2],
                                              tch[:], OP.mult)
                return h_new, i_h

            from concourse.tile import add_dep_helper

            # LSTM-B consumes hA with an emission skew of 2 iterations:
            # B(it-2) still reads exactly hA(it-2), but all its inputs are a
            # full period old, so the scheduler can float B's work freely.
            ha_hist = {}
            for it in range(T + 2):
                fa = fb = None
                if it < T:
                    ci, tl = divmod(it, CHUNK_T)
                    xr = xch[ci][:, tl * BC:(tl + 1) * BC]
                    fa = cell_front("a", ha, ca, xr)
                if fa is not None:
                    ha, iha = cell_tail("a", fa[0], fa[1])
                    ca = fa[1]
                    ha_hist[it] = ha
                if it >= 2:
                    fb = cell_front("b", hb, cb, ha_hist.pop(it - 2)[:])
                    if fa is not None:
                        # keep sigma_B off the A-chain: order it after tg_A
                        add_dep_helper(fb[2].ins, fa[3].ins, False,
                                       "act-order")
                    hb, _ = cell_tail("b", fb[0], fb[1])
                    cb = fb[1]

            # ---- dense tail ----
            # zcat [74, BC]: hB at rows 0:40, y at rows 64:74 (wc re-packed)
            zcat = spool.tile([74, BC], BF16, name="zcat")
            nc.gpsimd.memset(zcat[:], 0.0)
            nc.vector.tensor_copy(zcat[0:40, :], hb[:])

            ps1 = psum.tile([D, BC], F32, name="ps1", tag="zza")
            nc.tensor.matmul(ps1[:], W["wg"][:], ftile[:],
                             start=True, stop=True)
            y1 = spool.tile([D, BC], BF16, name="y1")
            nc.scalar.activation(y1[:], ps1[:], AF.Tanh, bias=W["bg"][:])

            ps2 = psum.tile([D, BC], F32, name="ps2", tag="gpb")
            nc.tensor.matmul(ps2[:], W["wh"][:], y1[:], start=True, stop=True)
            nc.scalar.activation(zcat[64:74, :], ps2[:], AF.Tanh,
                                 bias=W["bh"][:])

            ps3 = psum.tile([2 * D, BC], F32, name="ps3", tag="zza")
            nc.tensor.matmul(ps3[:], W["wc"][:], zcat[:], start=True,
                             stop=True)
            c1 = spool.tile([2 * D, BC], BF16, name="c1")
            nc.scalar.activation(c1[:], ps3[:], AF.Relu, bias=W["bc2"][:])

            ps4 = psum.tile([D, BC], F32, name="ps4", tag="gpb")
            nc.tensor.matmul(ps4[:], W["wd"][:], c1[:], start=True, stop=True)
            d1 = spool.tile([D, BC], BF16, name="d1")
            nc.scalar.activation(d1[:], ps4[:], AF.Relu, bias=W["bd"][:])

            ps5 = psum.tile([1, BC], F32, name="ps5", tag="zza")
            nc.tensor.matmul(ps5[:], W["wo"][:], d1[:], start=True, stop=True)
            osb = spool.tile([1, BC], F32, name="osb")
            nc.scalar.activation(osb[:], ps5[:], AF.Sigmoid, bias=W["bo"][:])

            nc.sync.dma_start(out_dram[:], osb[:])

    nc.compile()
    return nc


@functools.lru_cache(maxsize=2)
def _program(T, BC, use_bias_b):
    return _build_program(T, BC, use_bias_b)


def _prep_shared(Wa_k, Wa_r, ba, Wb_k, Wb_r, bb, Wg, bg, Wh, bh, Wc, bc, Wd,
                 bd, Wo, bo):
    zeros = np.zeros(160, np.float32)
    wc_re = np.zeros((74, 2 * D), np.float32)
    wc_re[0:40] = np.asarray(Wc, np.float32)[0:40]
    wc_re[64:74] = np.asarray(Wc, np.float32)[40:50]
    return {
        "wa_x_if": _wpair(Wa_k, ba, _I, _Fg, F, True),
        "wa_x_og": _wpair(Wa_k, ba, _O, _G, F, True),
        "wa_h_if": _wpair(Wa_r, zeros, _I, _Fg, H, False),
        "wa_h_og": _wpair(Wa_r, zeros, _O, _G, H, False),
        "wb_k_if": _wpair(Wb_k, zeros, _I, _Fg, H, False),
        "wb_k_og": _wpair(Wb_k, zeros, _O, _G, H, False),
        "wb_r_if": _wpair(Wb_r, zeros, _I, _Fg, H, False),
        "wb_r_og": _wpair(Wb_r, zeros, _O, _G, H, False),
        "bb_if": _wpair(np.zeros((0, 160), np.float32), bb, _I, _Fg, 0, True),
        "bb_og": _wpair(np.zeros((0, 160), np.float32), bb, _O, _G, 0, True),
        "wg": _bf(Wg), "wh": _bf(Wh), "wc": _bf(wc_re), "wd": _bf(Wd),
        "wo": _bf(Wo),
        "bg": _f32c(np.asarray(bg)[:, None]),
        "bh": _f32c(np.asarray(bh)[:, None]),
        "bc2": _f32c(np.asarray(bc)[:, None]),
        "bd": _f32c(np.asarray(bd)[:, None]),
        "bo": _f32c(np.asarray(bo)[:, None]),
    }


def _prep_seq(seq, T, BC, CHUNK_T):
    n_chunks = T // CHUNK_T
    arr = np.asarray(seq, np.float32).reshape(NCORES, BC, n_chunks, CHUNK_T, F)
    arr = arr.transpose(0, 2, 4, 3, 1)  # [core, chunk, F, CHUNK_T, BC]
    arr = arr.reshape(NCORES, n_chunks, F, CHUNK_T * BC)
    onesrow = np.ones((NCORES, n_chunks, 1, CHUNK_T * BC), np.float32)
    return _bf(np.concatenate([arr, onesrow], axis=2))


def kernel(seq, feat, Wa_k, Wa_r, ba, Wb_k, Wb_r, bb, Wg, bg, Wh, bh, Wc, bc,
           Wd, bd, Wo, bo, _trace=False):
    seq = np.asarray(seq)
    feat = np.asarray(feat)
    B, T, _ = seq.shape
    assert B % NCORES == 0
    BC = B // NCORES
    CHUNK_T = min(T, 128)

    use_bias_b = bool(np.any(np.asarray(bb)))
    nc = _program(T, BC, use_bias_b)

    shared = _prep_shared(Wa_k, Wa_r, ba, Wb_k, Wb_r, bb, Wg, bg, Wh, bh, Wc,
                          bc, Wd, bd, Wo, bo)
    xt = _prep_seq(seq, T, BC, CHUNK_T)
    featc = np.asarray(feat, np.float32).reshape(NCORES, BC, F)

    in_maps = []
    for c in range(NCORES):
        m = dict(shared)
        m["xt"] = xt[c]
        m["featT"] = _bf(featc[c].T)
        in_maps.append(m)

    res = run_bass_kernel_spmd(nc, in_maps, core_ids=list(range(NCORES)),
                               trace=_trace)
    out = np.concatenate([res.results[c]["out"][0] for c in range(NCORES)])
    out = out.astype(np.float32).reshape(B, 1)
    if _trace:
        kernel.last_results = res
    return out

